# revision 7
# baseline (speedup 1.0000x reference)
"""GaussianFC Trainium2 kernel.

out = relu(x @ W + bias),  W[i, o] = amp[i] * exp(-(o - mu[i])^2 / (2 sigma[i]^2))

Strategy (8 NeuronCores, out_features sharded, 1024 cols/core):
- Banded weights: sigma ~ 10 makes W effectively zero outside |o - mu| ~ 45.
  Host sorts rows by mu; each 128-col output block reads only the 256
  nearest (in mu) input rows, sliced at arbitrary (unaligned) offsets.
- z = (sc*(o - mu))^2 is quadratic in o, so each [128, 128] z tile is a
  rank-3 outer product: a K=8 bf16 matmul on PE against a fixed basis
  {o^2_hi, o^2_lo, o, 1} with hi/lo-split per-row coefficients (exact to
  ~5e-3 in z). This removes all per-tile DVE/ACT synthesis work.
- W = Exp(-z) runs as one parameter-free ACT op per 4-ktile group,
  PSUM -> SBUF bf16 (the only transcendental; ACT is the ceiling).
- Main matmuls keep W stationary (lhs) and stream x (64 moving rows,
  bf16): out^T[o, b] accumulates in PSUM; relu (+bias) on DVE/Pool;
  output leaves in SBUF-mirrored DRAM layout, host undoes the transpose.
- PE p-state: dummy matmuls fill the ~2.4us input-DMA latency window so
  real matmuls run at full clock.
"""
import numpy as np
from contextlib import ExitStack

import ml_dtypes

import concourse.bacc as bacc
import concourse.bass as bass
import concourse.mybir as mybir
import concourse.tile as tile
from concourse import bass_utils

f32 = mybir.dt.float32
bf16 = mybir.dt.bfloat16
AF = mybir.ActivationFunctionType
ALU = mybir.AluOpType
BF = ml_dtypes.bfloat16

NCORES = 8
BATCH = 64
IN_F = 8192
OUT_F = 8192
PER_CORE = OUT_F // NCORES  # 1024
NO = 128                    # output cols per block
B = PER_CORE // NO          # 8 blocks per core
KB = 256                    # band rows per block
NKT = B * 2                 # 16 k-tiles per core
GROUPS = 4                  # 4 k-tiles (2 blocks) per Exp group
NBASIS = 8                  # quadratic basis rows (7 used + 1 pad)

# ---- tuning knobs ----
NWARM_BIG = 5    # PE warmup matmuls with 512 moving rows
NWARM_SMALL = 2  # trailing warmup matmuls with 128 moving rows
GROUP_KT = (2, 6, 6, 2)  # k-tiles per Exp group (even: whole blocks)
ZBUFS = 2
WBUFS = 3
OBUFS = 2


def _build_program(has_bias):
    nc = bacc.Bacc("TRN2", target_bir_lowering=False, debug=False,
                   num_devices=NCORES)

    xt_d = nc.dram_tensor("xt", [128, NKT * BATCH], bf16,
                          kind="ExternalInput").ap()
    par_d = nc.dram_tensor("par", [NBASIS, NKT * NO + NO], bf16,
                           kind="ExternalInput").ap()
    bias_d = nc.dram_tensor("biasv", [128, B], f32,
                            kind="ExternalInput").ap()
    out_d = nc.dram_tensor("out", [128, B * BATCH], f32,
                           kind="ExternalOutput").ap()
    junk_d = nc.dram_tensor("junk", [1, 8], f32, kind="ExternalOutput").ap()

    gk = list(GROUP_KT)
    assert sum(gk) == NKT and all(k % 2 == 0 for k in gk)
    gs = [sum(gk[:i]) for i in range(len(gk) + 1)]  # ktile offsets

    with tile.TileContext(nc) as tc, ExitStack() as ctx:
        cpool = ctx.enter_context(tc.tile_pool(name="const", bufs=1))
        wpool = ctx.enter_context(tc.tile_pool(name="wts", bufs=WBUFS))
        spool = ctx.enter_context(tc.tile_pool(name="stage", bufs=len(gk)))
        zpool = ctx.enter_context(tc.tile_pool(name="zq", bufs=ZBUFS,
                                               space="PSUM"))
        dpool = ctx.enter_context(tc.tile_pool(name="dummy", bufs=1,
                                               space="PSUM"))
        opool = ctx.enter_context(tc.tile_pool(name="acc", bufs=OBUFS,
                                               space="PSUM"))

        t_par = cpool.tile([NBASIS, NKT * NO + NO], bf16, tag="par")
        nc.sync.dma_start(t_par[:], par_d)
        t_xt = cpool.tile([128, NKT * BATCH], bf16, tag="xt")
        nc.sync.dma_start(t_xt[:], xt_d)
        t_bias = cpool.tile([128, B], f32, tag="bias")
        nc.sync.dma_start(t_bias[:], bias_d)

        basis = t_par[:, NKT * NO: NKT * NO + NO]

        # PE warmup: keep the tensor engine continuously busy through the
        # input-DMA latency window so real matmuls run at full p-state.
        t_zero = cpool.tile([2, 512], bf16, tag="zeros")
        nc.vector.memset(t_zero[:], 0)
        dp = dpool.tile([128, 512], f32, tag="dp")
        for w in range(NWARM_BIG):
            nc.tensor.matmul(dp[:], t_zero[:, :128], t_zero[:],
                             start=True, stop=True)
        for w in range(NWARM_SMALL):
            nc.tensor.matmul(dp[:, :128], t_zero[:, :128], t_zero[:, :128],
                             start=True, stop=True)
        # Keep the warmup matmuls live: fold one value out to DRAM.
        t_junk = cpool.tile([1, 8], f32, tag="junk")
        nc.vector.tensor_scalar_max(t_junk[:], dp[:1, :8], 0.0)
        nc.sync.dma_start(junk_d, t_junk[:])

        def z_group(g):
            nkt = gk[g]
            zp = zpool.tile([128, nkt * NO], f32, tag="z")
            for t in range(nkt):
                jt = gs[g] + t
                nc.tensor.matmul(zp[:, t * NO:(t + 1) * NO],
                                 t_par[:, jt * NO:(jt + 1) * NO],
                                 basis, start=True, stop=True)
            return zp

        def exp_group(g, zp):
            wt = wpool.tile([128, gk[g] * NO], bf16, tag="w")
            nc.scalar.activation(wt[:], zp[:], AF.Exp, bias=0.0, scale=-1.0)
            return wt

        def mm_group(g, wt):
            og = opool.tile([128, (gk[g] // 2) * BATCH], f32, tag="og")
            for t in range(gk[g]):
                jt = gs[g] + t
                jl = t // 2
                nc.tensor.matmul(og[:, jl * BATCH:(jl + 1) * BATCH],
                                 wt[:, t * NO:(t + 1) * NO],
                                 t_xt[:, jt * BATCH:(jt + 1) * BATCH],
                                 start=(t % 2 == 0), stop=(t % 2 == 1))
            return og

        def relu_dma_group(g, og):
            # group g covers blocks gs[g]//2 .. gs[g+1]//2
            j0, j1 = gs[g] // 2, gs[g + 1] // 2
            sg = spool.tile([128, (j1 - j0) * BATCH], f32, tag="sg")
            if has_bias:
                for j in range(j0, j1):
                    jl = j - j0
                    nc.vector.tensor_scalar(sg[:, jl * BATCH:(jl + 1) * BATCH],
                                            og[:, jl * BATCH:(jl + 1) * BATCH],
                                            t_bias[:, j:j + 1], 0.0,
                                            ALU.add, ALU.max)
            else:
                nc.vector.tensor_scalar_max(sg[:], og[:], 0.0)
            # ACT.SEQ is free after the last Exp; SP queues the rest in-order
            eng = nc.scalar if g == len(gk) - 1 else nc.sync
            eng.dma_start(out_d[:, j0 * BATCH:j1 * BATCH], sg[:])

        # Interleave so PE never stalls on ACT.
        NG = len(gk)
        zps = [None] * NG
        zps[0] = z_group(0)
        zps[1] = z_group(1)
        for g in range(NG):
            wt = exp_group(g, zps[g])
            og = mm_group(g, wt)
            if g + 2 < NG:
                zps[g + 2] = z_group(g + 2)
            relu_dma_group(g, og)

    nc.compile()
    return nc


_PROG_CACHE = {}


def _prepare(x, mu, sigma, amplitude, bias):
    """Host-side packing: sort by mu, pick per-block bands, build the
    hi/lo-split quadratic coefficients and SBUF-mirrored input maps."""
    mu_f = np.asarray(mu, dtype=np.float64).ravel()
    sg_f = np.asarray(sigma, dtype=np.float64).ravel()
    am_f = np.asarray(amplitude, dtype=np.float64).ravel()
    perm = np.argsort(mu_f, kind="stable")
    mus = mu_f[perm]
    sgs = sg_f[perm]
    ams = am_f[perm]
    xp = np.ascontiguousarray(np.asarray(x, dtype=np.float32)[:, perm])
    if not np.allclose(ams, 1.0):
        xp = xp * ams[None, :].astype(np.float32)
    x_bf = xp.astype(BF)

    nblk = NCORES * B
    centers = np.arange(nblk, dtype=np.float64) * NO + NO / 2.0
    starts = np.clip(np.searchsorted(mus, centers) - KB // 2, 0, IN_F - KB)
    rows = starts[:, None] + np.arange(KB)[None, :]          # [nblk, KB]

    sc = 1.0 / (np.sqrt(2.0) * np.maximum(sgs[rows], 1e-30))  # [nblk, KB]
    v = sc * (mus[rows] - centers[:, None])
    A = sc * sc
    Bc = -2.0 * sc * v
    C = v * v

    def hilo(a):
        hi = a.astype(BF).astype(np.float64)
        lo = (a - hi).astype(BF)
        return hi.astype(BF), lo

    Ah, Al = hilo(A)
    Bh, Bl = hilo(Bc)
    Ch, Cl = hilo(C)
    # lhs rows pair with basis rows {o2h, o2h, o2l, o, o, 1, 1, 0}
    lhs = np.stack([Ah, Al, Ah, Bh, Bl, Ch, Cl,
                    np.zeros_like(Ah)], axis=1)              # [nblk, 8, KB]

    o_rel = np.arange(NO, dtype=np.float64) - NO / 2.0
    o2 = o_rel * o_rel
    r0h = o2.astype(BF).astype(np.float64)
    r0l = (o2 - r0h).astype(BF)
    basis = np.stack([r0h.astype(BF), r0h.astype(BF), r0l,
                      o_rel.astype(BF), o_rel.astype(BF),
                      np.ones(NO, BF), np.ones(NO, BF),
                      np.zeros(NO, BF)])                     # [8, NO]

    bias_v = np.asarray(bias, dtype=np.float32).ravel()
    has_bias = bool(np.any(bias_v != 0.0))

    # x gathered per block: [BATCH, nblk, KB] -> per-core xt
    xg = x_bf[:, rows]                                       # [64, nblk, 256]

    in_maps = []
    for c in range(NCORES):
        blk = slice(c * B, (c + 1) * B)
        # par: 16 lhs tiles [8, 128] + basis [8, 128]
        lh = lhs[blk].reshape(B, NBASIS, 2, NO)              # [8blk, 8, 2, 128]
        par = np.empty((NBASIS, NKT * NO + NO), dtype=BF)
        par[:, :NKT * NO] = lh.transpose(1, 0, 2, 3).reshape(NBASIS, NKT * NO)
        par[:, NKT * NO:] = basis
        # xt: [128, NKT*BATCH], col jt*64+b = x[b, rows[jg, (jt%2)*128+p]]
        xc = xg[:, blk].reshape(BATCH, B, 2, NO)             # [64, 8, 2, 128]
        xt = np.ascontiguousarray(
            xc.transpose(3, 1, 2, 0).reshape(128, NKT * BATCH))
        bm = np.ascontiguousarray(
            bias_v[c * PER_CORE:(c + 1) * PER_CORE].reshape(B, NO).T)
        in_maps.append({"xt": xt, "par": par, "biasv": bm})
    return in_maps, has_bias


def kernel(x, mu, sigma, amplitude, bias, _trace=False):
    in_maps, has_bias = _prepare(x, mu, sigma, amplitude, bias)
    if has_bias not in _PROG_CACHE:
        _PROG_CACHE[has_bias] = _build_program(has_bias)
    nc = _PROG_CACHE[has_bias]
    res = bass_utils.run_bass_kernel_spmd(nc, in_maps, list(range(NCORES)),
                                          trace=_trace)
    out = np.empty((BATCH, OUT_F), dtype=np.float32)
    for c in range(NCORES):
        # [128, B*BATCH] -> out[b, c*1024 + j*128 + p]
        arr = res.results[c]["out"].reshape(128, B, BATCH)
        out[:, c * PER_CORE:(c + 1) * PER_CORE] = \
            arr.transpose(2, 1, 0).reshape(BATCH, PER_CORE)
    if _trace:
        kernel._last = res
    return out


# revision 8
# speedup vs baseline: 1.0677x; 1.0677x over previous
"""GaussianFC Trainium2 kernel.

out = relu(x @ W + bias),  W[i, o] = amp[i] * exp(-(o - mu[i])^2 / (2 sigma[i]^2))

Strategy (8 NeuronCores, out_features sharded, 1024 cols/core):
- Banded weights: sigma ~ 10 makes W effectively zero outside |o - mu| ~ 45.
  Host sorts rows by mu; each 128-col output block reads only the 256
  nearest (in mu) input rows, sliced at arbitrary (unaligned) offsets.
- z = (sc*(o - mu))^2 is quadratic in o, so each [128, 128] z tile is a
  rank-3 outer product: a K=8 bf16 matmul on PE against a fixed basis
  {o^2_hi, o^2_lo, o, 1} with hi/lo-split per-row coefficients (exact to
  ~5e-3 in z). This removes all per-tile DVE/ACT synthesis work.
- W = Exp(-z) runs as one parameter-free ACT op per 4-ktile group,
  PSUM -> SBUF bf16 (the only transcendental; ACT is the ceiling).
- Main matmuls keep W stationary (lhs) and stream x (64 moving rows,
  bf16): out^T[o, b] accumulates in PSUM; relu (+bias) on DVE/Pool;
  output leaves in SBUF-mirrored DRAM layout, host undoes the transpose.
- PE p-state: dummy matmuls fill the ~2.4us input-DMA latency window so
  real matmuls run at full clock.
"""
import numpy as np
from contextlib import ExitStack

import ml_dtypes

import concourse.bacc as bacc
import concourse.bass as bass
import concourse.mybir as mybir
import concourse.tile as tile
from concourse import bass_utils

f32 = mybir.dt.float32
bf16 = mybir.dt.bfloat16
AF = mybir.ActivationFunctionType
ALU = mybir.AluOpType
BF = ml_dtypes.bfloat16

NCORES = 8
BATCH = 64
IN_F = 8192
OUT_F = 8192
PER_CORE = OUT_F // NCORES  # 1024
NO = 128                    # output cols per block
B = PER_CORE // NO          # 8 blocks per core
KB = 256                    # band rows per block
NKT = B * 2                 # 16 k-tiles per core
GROUPS = 4                  # 4 k-tiles (2 blocks) per Exp group
NBASIS = 8                  # quadratic basis rows (7 used + 1 pad)

# ---- tuning knobs ----
NWARM_BIG = 3    # PE warmup matmuls with 512 moving rows
NWARM_SMALL = 2  # trailing warmup matmuls with 128 moving rows
GROUP_KT = (2, 6, 6, 2)  # k-tiles per Exp group (even: whole blocks)
ZBUFS = 2
WBUFS = 3
OBUFS = 2


def _build_program(has_bias):
    nc = bacc.Bacc("TRN2", target_bir_lowering=False, debug=False,
                   num_devices=NCORES)

    xt_d = nc.dram_tensor("xt", [128, NKT * BATCH], bf16,
                          kind="ExternalInput").ap()
    par_d = nc.dram_tensor("par", [NBASIS, NKT * NO + NO], bf16,
                           kind="ExternalInput").ap()
    bias_d = nc.dram_tensor("biasv", [128, B], f32,
                            kind="ExternalInput").ap()
    out_d = nc.dram_tensor("out", [128, B * BATCH], f32,
                           kind="ExternalOutput").ap()

    gk = list(GROUP_KT)
    assert sum(gk) == NKT and all(k % 2 == 0 for k in gk)
    gs = [sum(gk[:i]) for i in range(len(gk) + 1)]  # ktile offsets

    with tile.TileContext(nc) as tc, ExitStack() as ctx:
        cpool = ctx.enter_context(tc.tile_pool(name="const", bufs=1))
        wpool = ctx.enter_context(tc.tile_pool(name="wts", bufs=WBUFS))
        spool = ctx.enter_context(tc.tile_pool(name="stage", bufs=len(gk)))
        zpool = ctx.enter_context(tc.tile_pool(name="zq", bufs=ZBUFS,
                                               space="PSUM"))
        dpool = ctx.enter_context(tc.tile_pool(name="dummy", bufs=1,
                                               space="PSUM"))
        opool = ctx.enter_context(tc.tile_pool(name="acc", bufs=OBUFS,
                                               space="PSUM"))

        t_par = cpool.tile([NBASIS, NKT * NO + NO], bf16, tag="par")
        nc.sync.dma_start(t_par[:], par_d)
        t_xt = cpool.tile([128, NKT * BATCH], bf16, tag="xt")
        nc.sync.dma_start(t_xt[:], xt_d)
        t_bias = cpool.tile([128, B], f32, tag="bias")
        nc.sync.dma_start(t_bias[:], bias_d)

        basis = t_par[:, NKT * NO: NKT * NO + NO]

        # PE warmup: keep the tensor engine continuously busy through the
        # input-DMA latency window so real matmuls run at full p-state.
        t_zero = cpool.tile([2, 512], bf16, tag="zeros")
        nc.vector.memset(t_zero[:], 0)
        dp = dpool.tile([128, 512], f32, tag="dp")
        for w in range(NWARM_BIG):
            nc.tensor.matmul(dp[:], t_zero[:, :128], t_zero[:],
                             start=True, stop=True)
        for w in range(NWARM_SMALL):
            nc.tensor.matmul(dp[:, :128], t_zero[:, :128], t_zero[:, :128],
                             start=True, stop=True)

        def z_group(g):
            nkt = gk[g]
            zp = zpool.tile([128, nkt * NO], f32, tag="z")
            for t in range(nkt):
                jt = gs[g] + t
                nc.tensor.matmul(zp[:, t * NO:(t + 1) * NO],
                                 t_par[:, jt * NO:(jt + 1) * NO],
                                 basis, start=True, stop=True)
            return zp

        def exp_group(g, zp):
            wt = wpool.tile([128, gk[g] * NO], bf16, tag="w")
            nc.scalar.activation(wt[:], zp[:], AF.Exp, bias=0.0, scale=-1.0)
            return wt

        def mm_group(g, wt):
            og = opool.tile([128, (gk[g] // 2) * BATCH], f32, tag="og")
            for t in range(gk[g]):
                jt = gs[g] + t
                jl = t // 2
                nc.tensor.matmul(og[:, jl * BATCH:(jl + 1) * BATCH],
                                 wt[:, t * NO:(t + 1) * NO],
                                 t_xt[:, jt * BATCH:(jt + 1) * BATCH],
                                 start=(t % 2 == 0), stop=(t % 2 == 1))
            return og

        def relu_dma_group(g, og):
            # group g covers blocks gs[g]//2 .. gs[g+1]//2
            j0, j1 = gs[g] // 2, gs[g + 1] // 2
            sg = spool.tile([128, (j1 - j0) * BATCH], f32, tag="sg")
            if has_bias:
                for j in range(j0, j1):
                    jl = j - j0
                    nc.vector.tensor_scalar(sg[:, jl * BATCH:(jl + 1) * BATCH],
                                            og[:, jl * BATCH:(jl + 1) * BATCH],
                                            t_bias[:, j:j + 1], 0.0,
                                            ALU.add, ALU.max)
            else:
                nc.vector.tensor_scalar_max(sg[:], og[:], 0.0)
            # ACT.SEQ is free after the last Exp; SP queues the rest in-order
            eng = nc.scalar if g == len(gk) - 1 else nc.sync
            eng.dma_start(out_d[:, j0 * BATCH:j1 * BATCH], sg[:])

        # Interleave so PE never stalls on ACT.
        NG = len(gk)
        zps = [None] * NG
        zps[0] = z_group(0)
        zps[1] = z_group(1)
        for g in range(NG):
            wt = exp_group(g, zps[g])
            og = mm_group(g, wt)
            if g + 2 < NG:
                zps[g + 2] = z_group(g + 2)
            relu_dma_group(g, og)

    nc.compile()
    return nc


_PROG_CACHE = {}


def _prepare(x, mu, sigma, amplitude, bias):
    """Host-side packing: sort by mu, pick per-block bands, build the
    hi/lo-split quadratic coefficients and SBUF-mirrored input maps."""
    mu_f = np.asarray(mu, dtype=np.float64).ravel()
    sg_f = np.asarray(sigma, dtype=np.float64).ravel()
    am_f = np.asarray(amplitude, dtype=np.float64).ravel()
    perm = np.argsort(mu_f, kind="stable")
    mus = mu_f[perm]
    sgs = sg_f[perm]
    ams = am_f[perm]
    xp = np.ascontiguousarray(np.asarray(x, dtype=np.float32)[:, perm])
    if not np.allclose(ams, 1.0):
        xp = xp * ams[None, :].astype(np.float32)
    x_bf = xp.astype(BF)

    nblk = NCORES * B
    centers = np.arange(nblk, dtype=np.float64) * NO + NO / 2.0
    starts = np.clip(np.searchsorted(mus, centers) - KB // 2, 0, IN_F - KB)
    rows = starts[:, None] + np.arange(KB)[None, :]          # [nblk, KB]

    sc = 1.0 / (np.sqrt(2.0) * np.maximum(sgs[rows], 1e-30))  # [nblk, KB]
    v = sc * (mus[rows] - centers[:, None])
    A = sc * sc
    Bc = -2.0 * sc * v
    C = v * v

    def hilo(a):
        hi = a.astype(BF).astype(np.float64)
        lo = (a - hi).astype(BF)
        return hi.astype(BF), lo

    Ah, Al = hilo(A)
    Bh, Bl = hilo(Bc)
    Ch, Cl = hilo(C)
    # lhs rows pair with basis rows {o2h, o2h, o2l, o, o, 1, 1, 0}
    lhs = np.stack([Ah, Al, Ah, Bh, Bl, Ch, Cl,
                    np.zeros_like(Ah)], axis=1)              # [nblk, 8, KB]

    o_rel = np.arange(NO, dtype=np.float64) - NO / 2.0
    o2 = o_rel * o_rel
    r0h = o2.astype(BF).astype(np.float64)
    r0l = (o2 - r0h).astype(BF)
    basis = np.stack([r0h.astype(BF), r0h.astype(BF), r0l,
                      o_rel.astype(BF), o_rel.astype(BF),
                      np.ones(NO, BF), np.ones(NO, BF),
                      np.zeros(NO, BF)])                     # [8, NO]

    bias_v = np.asarray(bias, dtype=np.float32).ravel()
    has_bias = bool(np.any(bias_v != 0.0))

    # x gathered per block: [BATCH, nblk, KB] -> per-core xt
    xg = x_bf[:, rows]                                       # [64, nblk, 256]

    in_maps = []
    for c in range(NCORES):
        blk = slice(c * B, (c + 1) * B)
        # par: 16 lhs tiles [8, 128] + basis [8, 128]
        lh = lhs[blk].reshape(B, NBASIS, 2, NO)              # [8blk, 8, 2, 128]
        par = np.empty((NBASIS, NKT * NO + NO), dtype=BF)
        par[:, :NKT * NO] = lh.transpose(1, 0, 2, 3).reshape(NBASIS, NKT * NO)
        par[:, NKT * NO:] = basis
        # xt: [128, NKT*BATCH], col jt*64+b = x[b, rows[jg, (jt%2)*128+p]]
        xc = xg[:, blk].reshape(BATCH, B, 2, NO)             # [64, 8, 2, 128]
        xt = np.ascontiguousarray(
            xc.transpose(3, 1, 2, 0).reshape(128, NKT * BATCH))
        bm = np.ascontiguousarray(
            bias_v[c * PER_CORE:(c + 1) * PER_CORE].reshape(B, NO).T)
        in_maps.append({"xt": xt, "par": par, "biasv": bm})
    return in_maps, has_bias


def kernel(x, mu, sigma, amplitude, bias, _trace=False):
    in_maps, has_bias = _prepare(x, mu, sigma, amplitude, bias)
    if has_bias not in _PROG_CACHE:
        _PROG_CACHE[has_bias] = _build_program(has_bias)
    nc = _PROG_CACHE[has_bias]
    res = bass_utils.run_bass_kernel_spmd(nc, in_maps, list(range(NCORES)),
                                          trace=_trace)
    out = np.empty((BATCH, OUT_F), dtype=np.float32)
    for c in range(NCORES):
        # [128, B*BATCH] -> out[b, c*1024 + j*128 + p]
        arr = res.results[c]["out"].reshape(128, B, BATCH)
        out[:, c * PER_CORE:(c + 1) * PER_CORE] = \
            arr.transpose(2, 1, 0).reshape(BATCH, PER_CORE)
    if _trace:
        kernel._last = res
    return out


# revision 10
# speedup vs baseline: 1.0937x; 1.0244x over previous
"""GaussianFC Trainium2 kernel.

out = relu(x @ W + bias),  W[i, o] = amp[i] * exp(-(o - mu[i])^2 / (2 sigma[i]^2))

Strategy (8 NeuronCores, out_features sharded, 1024 cols/core):
- Banded weights: sigma ~ 10 makes W effectively zero outside |o - mu| ~ 45.
  Host sorts rows by mu; each 128-col output block reads only the 256
  nearest (in mu) input rows, sliced at arbitrary (unaligned) offsets.
- z = (sc*(o - mu))^2 is quadratic in o, so each [128, 128] z tile is a
  rank-3 outer product: a K=8 bf16 matmul on PE against a fixed basis
  {o^2_hi, o^2_lo, o, 1} with hi/lo-split per-row coefficients (exact to
  ~5e-3 in z). This removes all per-tile DVE/ACT synthesis work.
- W = Exp(-z) runs as one parameter-free ACT op per 4-ktile group,
  PSUM -> SBUF bf16 (the only transcendental; ACT is the ceiling).
- Main matmuls keep W stationary (lhs) and stream x (64 moving rows,
  bf16): out^T[o, b] accumulates in PSUM; relu (+bias) on DVE/Pool;
  output leaves in SBUF-mirrored DRAM layout, host undoes the transpose.
- PE p-state: dummy matmuls fill the ~2.4us input-DMA latency window so
  real matmuls run at full clock.
"""
import numpy as np
from contextlib import ExitStack

import ml_dtypes

import concourse.bacc as bacc
import concourse.bass as bass
import concourse.mybir as mybir
import concourse.tile as tile
from concourse import bass_utils

f32 = mybir.dt.float32
bf16 = mybir.dt.bfloat16
AF = mybir.ActivationFunctionType
ALU = mybir.AluOpType
BF = ml_dtypes.bfloat16

NCORES = 8
BATCH = 64
IN_F = 8192
OUT_F = 8192
PER_CORE = OUT_F // NCORES  # 1024
NO = 128                    # output cols per block
B = PER_CORE // NO          # 8 blocks per core
KB = 256                    # band rows per block
NKT = B * 2                 # 16 k-tiles per core
GROUPS = 4                  # 4 k-tiles (2 blocks) per Exp group
NBASIS = 8                  # quadratic basis rows (7 used + 1 pad)

# ---- tuning knobs ----
NWARM_BIG = 3    # PE warmup matmuls with 512 moving rows
NWARM_SMALL = 3  # trailing warmup matmuls with 128 moving rows
GROUP_KT = (2, 6, 6, 2)  # k-tiles per Exp group (even: whole blocks)
ZBUFS = 2
WBUFS = 3
OBUFS = 2


def _build_program(has_bias):
    nc = bacc.Bacc("TRN2", target_bir_lowering=False, debug=False,
                   num_devices=NCORES)

    xt_d = nc.dram_tensor("xt", [128, NKT * BATCH], bf16,
                          kind="ExternalInput").ap()
    par_d = nc.dram_tensor("par", [NBASIS, NKT * NO + NO], bf16,
                           kind="ExternalInput").ap()
    bias_d = nc.dram_tensor("biasv", [128, B], f32,
                            kind="ExternalInput").ap()
    out_d = nc.dram_tensor("out", [128, B * BATCH], f32,
                           kind="ExternalOutput").ap()

    gk = list(GROUP_KT)
    assert sum(gk) == NKT and all(k % 2 == 0 for k in gk)
    gs = [sum(gk[:i]) for i in range(len(gk) + 1)]  # ktile offsets

    with tile.TileContext(nc) as tc, ExitStack() as ctx:
        cpool = ctx.enter_context(tc.tile_pool(name="const", bufs=1))
        wpool = ctx.enter_context(tc.tile_pool(name="wts", bufs=WBUFS))
        spool = ctx.enter_context(tc.tile_pool(name="stage", bufs=len(gk)))
        zpool = ctx.enter_context(tc.tile_pool(name="zq", bufs=ZBUFS,
                                               space="PSUM"))
        dpool = ctx.enter_context(tc.tile_pool(name="dummy", bufs=1,
                                               space="PSUM"))
        opool = ctx.enter_context(tc.tile_pool(name="acc", bufs=OBUFS,
                                               space="PSUM"))

        t_par = cpool.tile([NBASIS, NKT * NO + NO], bf16, tag="par")
        nc.sync.dma_start(t_par[:], par_d)
        t_xt = cpool.tile([128, NKT * BATCH], bf16, tag="xt")
        nc.sync.dma_start(t_xt[:], xt_d)
        t_bias = cpool.tile([128, B], f32, tag="bias")
        nc.sync.dma_start(t_bias[:], bias_d)

        basis = t_par[:, NKT * NO: NKT * NO + NO]

        # PE warmup: keep the tensor engine continuously busy through the
        # input-DMA latency window so real matmuls run at full p-state.
        t_zero = cpool.tile([2, 512], bf16, tag="zeros")
        nc.gpsimd.memset(t_zero[:], 0)
        dp = dpool.tile([128, 512], f32, tag="dp")
        for w in range(NWARM_BIG):
            nc.tensor.matmul(dp[:], t_zero[:, :128], t_zero[:],
                             start=True, stop=True)
        for w in range(NWARM_SMALL):
            nc.tensor.matmul(dp[:, :128], t_zero[:, :128], t_zero[:, :128],
                             start=True, stop=True)

        def z_group(g):
            nkt = gk[g]
            zp = zpool.tile([128, nkt * NO], f32, tag="z")
            for t in range(nkt):
                jt = gs[g] + t
                nc.tensor.matmul(zp[:, t * NO:(t + 1) * NO],
                                 t_par[:, jt * NO:(jt + 1) * NO],
                                 basis, start=True, stop=True)
            return zp

        def exp_group(g, zp):
            wt = wpool.tile([128, gk[g] * NO], bf16, tag="w")
            nc.scalar.activation(wt[:], zp[:], AF.Exp, bias=0.0, scale=-1.0)
            return wt

        def mm_group(g, wt):
            og = opool.tile([128, (gk[g] // 2) * BATCH], f32, tag="og")
            for t in range(gk[g]):
                jt = gs[g] + t
                jl = t // 2
                nc.tensor.matmul(og[:, jl * BATCH:(jl + 1) * BATCH],
                                 wt[:, t * NO:(t + 1) * NO],
                                 t_xt[:, jt * BATCH:(jt + 1) * BATCH],
                                 start=(t % 2 == 0), stop=(t % 2 == 1))
            return og

        NG_ = len(gk)
        # last two groups share one staging tile and a single DMA issued
        # after the final relu, so only one HWDGE slot sits in the tail
        sg_tail = None

        def relu_dma_group(g, og):
            nonlocal sg_tail
            # group g covers blocks gs[g]//2 .. gs[g+1]//2
            j0, j1 = gs[g] // 2, gs[g + 1] // 2
            tail2 = g >= NG_ - 2
            if tail2:
                jt0 = gs[NG_ - 2] // 2   # first block of merged tail
                if sg_tail is None:
                    sg_tail = spool.tile([128, (B - jt0) * BATCH], f32,
                                         tag="sgt")
                sg = sg_tail[:, (j0 - jt0) * BATCH:(j1 - jt0) * BATCH]
            else:
                sg_own = spool.tile([128, (j1 - j0) * BATCH], f32, tag="sg")
                sg = sg_own[:]
            if has_bias:
                for j in range(j0, j1):
                    jl = j - j0
                    nc.vector.tensor_scalar(sg[:, jl * BATCH:(jl + 1) * BATCH],
                                            og[:, jl * BATCH:(jl + 1) * BATCH],
                                            t_bias[:, j:j + 1], 0.0,
                                            ALU.add, ALU.max)
            else:
                nc.vector.tensor_scalar_max(sg, og[:], 0.0)
            if g == NG_ - 1:
                # ACT.SEQ is free after the last Exp
                nc.scalar.dma_start(out_d[:, jt0 * BATCH:], sg_tail[:])
            elif not tail2:
                nc.sync.dma_start(out_d[:, j0 * BATCH:j1 * BATCH], sg)

        # Interleave so PE never stalls on ACT.
        NG = len(gk)
        zps = [None] * NG
        zps[0] = z_group(0)
        zps[1] = z_group(1)
        for g in range(NG):
            wt = exp_group(g, zps[g])
            og = mm_group(g, wt)
            if g + 2 < NG:
                zps[g + 2] = z_group(g + 2)
            relu_dma_group(g, og)

    nc.compile()
    return nc


_PROG_CACHE = {}


def _prepare(x, mu, sigma, amplitude, bias):
    """Host-side packing: sort by mu, pick per-block bands, build the
    hi/lo-split quadratic coefficients and SBUF-mirrored input maps."""
    mu_f = np.asarray(mu, dtype=np.float64).ravel()
    sg_f = np.asarray(sigma, dtype=np.float64).ravel()
    am_f = np.asarray(amplitude, dtype=np.float64).ravel()
    perm = np.argsort(mu_f, kind="stable")
    mus = mu_f[perm]
    sgs = sg_f[perm]
    ams = am_f[perm]
    xp = np.ascontiguousarray(np.asarray(x, dtype=np.float32)[:, perm])
    if not np.allclose(ams, 1.0):
        xp = xp * ams[None, :].astype(np.float32)
    x_bf = xp.astype(BF)

    nblk = NCORES * B
    centers = np.arange(nblk, dtype=np.float64) * NO + NO / 2.0
    starts = np.clip(np.searchsorted(mus, centers) - KB // 2, 0, IN_F - KB)
    rows = starts[:, None] + np.arange(KB)[None, :]          # [nblk, KB]

    sc = 1.0 / (np.sqrt(2.0) * np.maximum(sgs[rows], 1e-30))  # [nblk, KB]
    v = sc * (mus[rows] - centers[:, None])
    A = sc * sc
    Bc = -2.0 * sc * v
    C = v * v

    def hilo(a):
        hi = a.astype(BF).astype(np.float64)
        lo = (a - hi).astype(BF)
        return hi.astype(BF), lo

    Ah, Al = hilo(A)
    Bh, Bl = hilo(Bc)
    Ch, Cl = hilo(C)
    # lhs rows pair with basis rows {o2h, o2h, o2l, o, o, 1, 1, 0}
    lhs = np.stack([Ah, Al, Ah, Bh, Bl, Ch, Cl,
                    np.zeros_like(Ah)], axis=1)              # [nblk, 8, KB]

    o_rel = np.arange(NO, dtype=np.float64) - NO / 2.0
    o2 = o_rel * o_rel
    r0h = o2.astype(BF).astype(np.float64)
    r0l = (o2 - r0h).astype(BF)
    basis = np.stack([r0h.astype(BF), r0h.astype(BF), r0l,
                      o_rel.astype(BF), o_rel.astype(BF),
                      np.ones(NO, BF), np.ones(NO, BF),
                      np.zeros(NO, BF)])                     # [8, NO]

    bias_v = np.asarray(bias, dtype=np.float32).ravel()
    has_bias = bool(np.any(bias_v != 0.0))

    # x gathered per block: [BATCH, nblk, KB] -> per-core xt
    xg = x_bf[:, rows]                                       # [64, nblk, 256]

    in_maps = []
    for c in range(NCORES):
        blk = slice(c * B, (c + 1) * B)
        # par: 16 lhs tiles [8, 128] + basis [8, 128]
        lh = lhs[blk].reshape(B, NBASIS, 2, NO)              # [8blk, 8, 2, 128]
        par = np.empty((NBASIS, NKT * NO + NO), dtype=BF)
        par[:, :NKT * NO] = lh.transpose(1, 0, 2, 3).reshape(NBASIS, NKT * NO)
        par[:, NKT * NO:] = basis
        # xt: [128, NKT*BATCH], col jt*64+b = x[b, rows[jg, (jt%2)*128+p]]
        xc = xg[:, blk].reshape(BATCH, B, 2, NO)             # [64, 8, 2, 128]
        xt = np.ascontiguousarray(
            xc.transpose(3, 1, 2, 0).reshape(128, NKT * BATCH))
        bm = np.ascontiguousarray(
            bias_v[c * PER_CORE:(c + 1) * PER_CORE].reshape(B, NO).T)
        in_maps.append({"xt": xt, "par": par, "biasv": bm})
    return in_maps, has_bias


def kernel(x, mu, sigma, amplitude, bias, _trace=False):
    in_maps, has_bias = _prepare(x, mu, sigma, amplitude, bias)
    if has_bias not in _PROG_CACHE:
        _PROG_CACHE[has_bias] = _build_program(has_bias)
    nc = _PROG_CACHE[has_bias]
    res = bass_utils.run_bass_kernel_spmd(nc, in_maps, list(range(NCORES)),
                                          trace=_trace)
    out = np.empty((BATCH, OUT_F), dtype=np.float32)
    for c in range(NCORES):
        # [128, B*BATCH] -> out[b, c*1024 + j*128 + p]
        arr = res.results[c]["out"].reshape(128, B, BATCH)
        out[:, c * PER_CORE:(c + 1) * PER_CORE] = \
            arr.transpose(2, 1, 0).reshape(BATCH, PER_CORE)
    if _trace:
        kernel._last = res
    return out
